# revision 4
# baseline (speedup 1.0000x reference)
"""Causal attention for Trainium2, sequence-parallel over 8 NeuronCores — v2.

reference:
    q = x @ Wq.T ; k = x @ Wk.T ; v = x @ Wv.T      (biases are zero)
    scores = q @ k.T / sqrt(D) + mask
    out = softmax(scores, -1) @ v

Core c owns query tiles {c, c+8, ..., c+56} (cyclic by 128 rows).  Folding
the projections (A = Wq.T @ Wk / sqrt(D)):
    sT[k, q] = x[k, :] . y[q, :],   y = xq @ A
    out = ((p @ x) / rowsum(p)) @ Wv.T,   p = exp(sT + causal)
Scores are computed TRANSPOSED ([key-part, query-free]) so the exp output
IS the pv lhsT — no on-chip transposes of p at all.  x is supplied by the
host in bf16 twice (natural + transposed), so there are no on-chip x
transposes or casts either.  The 64MB mask never reaches the device: the
causal structure collapses to one per-core [128, 8, 128] additive strip
(0 / -1e9) applied to the diagonal query tile as an extra chained matmul
(identity @ strip) that initializes the first 128 score columns.

Softmax needs no max subtraction: scores are O(1) by construction and
masked entries exp to exactly 0.  Row sums come from a ones-column matmul
chain; z and l accumulate across key blocks in SBUF (f32).
"""

import sys
from contextlib import ExitStack, nullcontext

if "/opt/trn_rl_repo" not in sys.path:
    sys.path.insert(0, "/opt/trn_rl_repo")

import numpy as np
import ml_dtypes

import concourse.bass as bass
import concourse.tile as tile
from concourse import bacc, mybir
from concourse.bass_utils import run_bass_kernel_spmd
from concourse.masks import make_identity

F32 = mybir.dt.float32
F32R = mybir.dt.float32r
BF16 = mybir.dt.bfloat16
NPBF16 = ml_dtypes.bfloat16

N, D, NCORES = 8192, 512, 8
P = 128           # partitions
KB = 1024         # key-block size
Q = N // NCORES   # per-core query rows
NQT = Q // P      # q-tiles per core
NB = N // KB      # key blocks
TPB = KB // P     # key tiles per block
DC = D // P       # d chunks


def build(reps=1, trace_sim=False):
    inv_sqrt_d = 1.0 / float(np.sqrt(D))
    nc = bacc.Bacc("TRN2", target_bir_lowering=False, debug=False,
                   num_devices=NCORES)
    xt_d = nc.dram_tensor("xt", [P, DC, N], BF16, kind="ExternalInput").ap()
    xn_d = nc.dram_tensor("xn", [N, D], BF16, kind="ExternalInput").ap()
    xqt_d = nc.dram_tensor("xqt", [P, DC, Q], BF16, kind="ExternalInput").ap()
    wq_d = nc.dram_tensor("wq", [P, DC, D], BF16, kind="ExternalInput").ap()
    wk_d = nc.dram_tensor("wk", [P, DC, D], BF16, kind="ExternalInput").ap()
    wvt_d = nc.dram_tensor("wvt", [P, DC, D], BF16, kind="ExternalInput").ap()
    ms_d = nc.dram_tensor("mstrip", [P, TPB, P], BF16, kind="ExternalInput").ap()
    out_d = nc.dram_tensor("out", [Q, D], F32, kind="ExternalOutput").ap()

    # Alternate SBUF-writing copies between ACT and DVE to balance load.
    flip = [0]

    def copy(out, in_):
        flip[0] ^= 1
        if flip[0]:
            nc.scalar.copy(out=out, in_=in_)
        else:
            nc.vector.tensor_copy(out=out, in_=in_)

    with tile.TileContext(nc, trace_sim=trace_sim) as tc, ExitStack() as st:
        consts = st.enter_context(tc.tile_pool(name="consts", bufs=1))
        wts = st.enter_context(tc.tile_pool(name="wts", bufs=1))
        xt_p = st.enter_context(tc.tile_pool(name="xt", bufs=2))
        xn_p = st.enter_context(tc.tile_pool(name="xn", bufs=3))
        pt_p = st.enter_context(tc.tile_pool(name="pt", bufs=2))
        acc_p = st.enter_context(tc.tile_pool(name="acc", bufs=1))
        fin_p = st.enter_context(tc.tile_pool(name="fin", bufs=2))
        ps_s = st.enter_context(tc.tile_pool(name="ps_s", bufs=4, space="PSUM"))
        ps_z = st.enter_context(tc.tile_pool(name="ps_z", bufs=3, space="PSUM"))
        ps_l = st.enter_context(tc.tile_pool(name="ps_l", bufs=1, space="PSUM"))

        loop = tc.For_i(0, reps, 1) if reps > 1 else nullcontext()
        with loop:
            ident = consts.tile([P, P], F32, tag="ident")
            make_identity(nc, ident)
            ident_r = consts.tile([P, P], F32R, tag="ident_r")
            nc.vector.tensor_copy(out=ident_r, in_=ident)
            ones = consts.tile([P, 1], BF16, tag="ones")
            nc.vector.memset(ones, 1.0)

            # DMA issue order = landing order: A's weights first, then xqt
            # (yT), then the mask strip; wvt (finalize-only) is deferred
            # until after the first block loads.
            wq_s = wts.tile([P, DC, D], BF16, tag="wq")
            nc.gpsimd.dma_start(out=wq_s, in_=wq_d)
            wk_s = wts.tile([P, DC, D], BF16, tag="wk")
            nc.gpsimd.dma_start(out=wk_s, in_=wk_d)
            xqt_s = wts.tile([P, DC, Q], BF16, tag="xqt")
            nc.gpsimd.dma_start(out=xqt_s, in_=xqt_d)
            mstrip = wts.tile([P, TPB, P], BF16, tag="mstrip")
            nc.gpsimd.dma_start(out=mstrip, in_=ms_d)
            wvt_s = wts.tile([P, DC, D], BF16, tag="wvt")

            # ---- A[i, j] = sum_d Wq[d, i] Wk[d, j]  (1/sqrt(D) folded into
            # wq on the host) ----
            A_sb = wts.tile([P, DC, D], BF16, tag="A_sb")
            for ic in range(DC):
                ps = ps_z.tile([P, D], F32, tag="ps_z")
                for m in range(DC):
                    nc.tensor.matmul(ps, wq_s[:, m, ic * P:(ic + 1) * P],
                                     wk_s[:, m, :],
                                     start=(m == 0), stop=(m == DC - 1))
                copy(A_sb[:, ic, :], ps)

            # ---- yT[j, q] = sum_i A[i, j] xqT[i, q] ----
            # q-descending halves: the first processed key block (b=7) only
            # needs the tail of yT, so scoring starts before yT completes.
            yT = wts.tile([P, DC, Q], BF16, tag="yT")
            for qh in range(Q - 512, -1, -512):
                for jc in range(DC):
                    ps = ps_z.tile([P, D], F32, tag="ps_z")
                    for ic in range(DC):
                        nc.tensor.matmul(ps, A_sb[:, ic, jc * P:(jc + 1) * P],
                                         xqt_s[:, ic, qh:qh + 512],
                                         start=(ic == 0), stop=(ic == DC - 1))
                    copy(yT[:, jc, qh:qh + 512], ps)

            zacc = acc_p.tile([P, NQT, D], F32R, tag="zacc")
            lacc = acc_p.tile([P, NQT], F32, tag="lacc")

            def load(b):
                xT = xt_p.tile([P, DC, KB], BF16, tag="xt")
                nc.gpsimd.dma_start(out=xT, in_=xt_d[:, :, b * KB:(b + 1) * KB])
                xN = xn_p.tile([P, TPB, D], BF16, tag="xn")
                nc.gpsimd.dma_start(
                    out=xN, in_=xn_d[b * KB:(b + 1) * KB, :]
                    .rearrange("(t p) d -> p t d", p=P))
                return xT, xN

            mflip = [0]

            def scores(b, xT):
                W = (NQT - b) * P      # live query width (q-tiles [b, 8))
                pT = pt_p.tile([P, TPB, Q], BF16, tag="pt")
                for kt in range(TPB):
                    segs = [(0, min(W, D))]
                    if W > D:
                        segs.append((D, W - D))
                    for ch, cw in segs:
                        ps = ps_s.tile([P, D], F32, tag="ps_s")
                        for cc in range(DC):
                            nc.tensor.matmul(
                                ps[:, 0:cw],
                                xT[:, cc, kt * P:(kt + 1) * P],
                                yT[:, cc, b * P + ch:b * P + ch + cw],
                                start=(cc == 0), stop=(cc == DC - 1))
                        nc.scalar.activation(
                            out=pT[:, kt, ch:ch + cw], in_=ps[:, 0:cw],
                            func=mybir.ActivationFunctionType.Exp)
                    # causal 0/1 strip zeroes the dead part of the diagonal
                    # q-tile (exact: p*0 == exp(-1e9) == 0)
                    mflip[0] ^= 1
                    eng = nc.vector if mflip[0] else nc.gpsimd
                    eng.tensor_mul(out=pT[:, kt, 0:P], in0=pT[:, kt, 0:P],
                                   in1=mstrip[:, kt, :])
                return pT

            # finalize is split in two so its PE work pipelines between
            # consecutive pv chains: out = (z @ Wv.T) / l (row scale last).
            fin_state = {}

            def fin1(t):
                linv = fin_p.tile([P, 1], F32, tag="linv")
                nc.vector.reciprocal(linv, lacc[:, t:t + 1])
                ps_t = ps_z.tile([P, D], F32R, tag="ps_z")
                for ic in range(DC):
                    nc.tensor.transpose(ps_t[:, ic * P:(ic + 1) * P],
                                        zacc[:, t, ic * P:(ic + 1) * P],
                                        ident_r)
                znT = fin_p.tile([P, DC, P], BF16, tag="znT")
                copy(znT, ps_t.rearrange("p (i f) -> p i f", f=P))
                fin_state[t] = (linv, znT)

            def fin2(t):
                linv, znT = fin_state.pop(t)
                pso = ps_z.tile([P, D], F32, tag="ps_z")
                for cc in range(DC):
                    nc.tensor.matmul(pso, znT[:, cc, :], wvt_s[:, cc, :],
                                     start=(cc == 0), stop=(cc == DC - 1))
                ot = fin_p.tile([P, D], F32, tag="ot")
                nc.vector.tensor_scalar_mul(out=ot, in0=pso, scalar1=linv)
                nc.sync.dma_start(out=out_d[t * P:(t + 1) * P, :], in_=ot)

            def pv(b, pT, xN):
                for t in range(b, NQT):
                    j = t - b
                    psz = ps_z.tile([P, D], F32, tag="ps_z")
                    psl = ps_l.tile([P, 1], F32, tag="ps_l")
                    # z and l chains interleaved per key tile: adjacent
                    # matmuls share the same stationary pT slice
                    for kt in range(TPB):
                        w = pT[:, kt, j * P:(j + 1) * P]
                        nc.tensor.matmul(psz, w, xN[:, kt, :],
                                         start=(kt == 0), stop=(kt == TPB - 1))
                        nc.tensor.matmul(psl, w, ones,
                                         start=(kt == 0), stop=(kt == TPB - 1))
                    if b == t:
                        copy(zacc[:, t, :], psz)
                        copy(lacc[:, t:t + 1], psl)
                    else:
                        nc.vector.tensor_add(out=zacc[:, t, :],
                                             in0=zacc[:, t, :], in1=psz)
                        nc.vector.tensor_add(out=lacc[:, t:t + 1],
                                             in0=lacc[:, t:t + 1], in1=psl)
                    if b == 0:
                        if t >= 1:
                            fin1(t - 1)
                        if t >= 2:
                            fin2(t - 2)
                        if t == NQT - 1:
                            fin1(t)
                            fin2(t - 1)
                            fin2(t)

            # Descending blocks; DMA prefetched one block ahead; pv runs one
            # block behind scores so it never waits on the freshest exp.
            ld = load(NB - 1)
            nc.gpsimd.dma_start(out=wvt_s, in_=wvt_d)
            prev = None
            for b in range(NB - 1, -1, -1):
                ld_next = load(b - 1) if b > 0 else None
                pT = scores(b, ld[0])
                if prev is not None:
                    pv(b + 1, prev[0], prev[1])
                prev = (pT, ld[1])
                ld = ld_next
            pv(0, prev[0], prev[1])

    nc.compile()
    return nc


def core_rows(n, ncores, c):
    nt_global = n // P
    tiles = list(range(c, nt_global, ncores))
    return np.concatenate([np.arange(g * P, (g + 1) * P) for g in tiles])


def prepare_in_maps(x, mask, Wq, bq, Wk, bk, Wv, bv):
    x = np.asarray(x, np.float32)
    for b in (bq, bk, bv):
        assert not np.any(np.asarray(b)), "zero-bias fast path only"
    # cheap causal-mask verification on a sample of 128-row bands
    m = np.asarray(mask)
    idx = np.arange(N)
    for r in (0, 1, 4095, 8191, 2917):
        row = m[r]
        assert np.all(row[: r + 1] == 0.0) and np.all(row[r + 1:] <= -1e8), \
            "kernel specialized to the causal mask"
    f = np.ascontiguousarray
    xb = x.astype(NPBF16)
    xtb = f(x.T.astype(NPBF16).reshape(DC, P, N).transpose(1, 0, 2))
    wqb = f((np.asarray(Wq, np.float32) / np.sqrt(D)).astype(NPBF16)
            .reshape(DC, P, D).transpose(1, 0, 2))
    wkb = f(np.asarray(Wk, np.float32).astype(NPBF16)
            .reshape(DC, P, D).transpose(1, 0, 2))
    wvtb = f(np.asarray(Wv, np.float32).T.astype(NPBF16)
             .reshape(DC, P, D).transpose(1, 0, 2))
    k_in = np.arange(P)[:, None]          # key within tile (partition)
    q_in = np.arange(P)[None, :]          # query within tile (free)
    rows = [core_rows(N, NCORES, c) for c in range(NCORES)]
    in_maps = []
    for c in range(NCORES):
        ms = np.empty((P, TPB, P), np.float32)
        for kt in range(TPB):
            live = (c - kt) * P + q_in - k_in >= 0
            ms[:, kt, :] = np.where(live, 1.0, 0.0)
        xqtb = f(x[rows[c]].T.astype(NPBF16).reshape(DC, P, Q)
                 .transpose(1, 0, 2))
        in_maps.append({
            "xt": xtb, "xn": xb, "xqt": xqtb,
            "wq": wqb, "wk": wkb, "wvt": wvtb,
            "mstrip": ms.astype(NPBF16),
        })
    return in_maps, {"rows": rows}


_CACHED = {}


def kernel(x, mask, Wq, bq, Wk, bk, Wv, bv):
    x = np.asarray(x)
    in_maps, meta = prepare_in_maps(x, mask, Wq, bq, Wk, bk, Wv, bv)
    if "nc" not in _CACHED:
        _CACHED["nc"] = build()
    nc = _CACHED["nc"]
    res = run_bass_kernel_spmd(nc, in_maps, list(range(NCORES)))
    out = np.empty((x.shape[0], x.shape[1]), np.float32)
    for c in range(NCORES):
        out[meta["rows"][c]] = res.results[c]["out"]
    return out
